# revision 9
# baseline (speedup 1.0000x reference)
"""Trainium2 Bass kernel for nn_Attention_20461224198682 (v2: key compaction).

Multi-head attention (B=64, N=196, C=768, H=12, D=64) with relative position
bias and key masking. Data-parallel over batch across 8 NeuronCores (8
batches/core). All device compute in a transposed layout (feature dim on
partitions) so no on-device transposes are needed.

v2: the key mask zeroes ~50% of keys, so masked keys are dropped on the host
and each batch's keys are compacted into <=128 slots (max count for the
fixed inputs is 119). This halves the k/v projections, score and AV matmuls
and the softmax element ops. Padded key slots carry k=bias-only scores but
their rpe bias table entries are 0, so exp(s)*bias == 0 kills them exactly;
the v ones-column is scaled by the key-count mask so denominators are exact.

  qkv^T[o,t]  = Wqkv^T-as-lhsT @ x^T    (q on all 196 tokens; k,v on the
                                         compacted 128-slot key axis)
  s^T[m,n]    = k^T-as-lhsT @ q^T       (single 128-key chunk)
  p[m,n]      = exp(s^T) * exp(bias)^T  (bias gathered per batch on host,
                                         exponentiated, 0 on padded slots)
  o^T[d,n]    = v-as-lhsT @ p           (v rows 0 on padded slots; ones col
                                         gives the masked softmax denom)
  o^T[0:64]  /= denom
  out^T[o2,t] = Wproj^T-as-lhsT @ concat_h o^T   (+ proj bias incl. v bias)

Attention emission is software-pipelined (depth 2): score matmuls of calls
N+1,N+2 are emitted before the AV matmuls of call N so the in-order PE queue
never head-of-line blocks on the ACT exp -> DVE mul chain.
"""
import numpy as np
import ml_dtypes

B, N, C, H = 64, 196, 768, 12
D = C // H
SCALE = D ** -0.5
TABLE = 729
NCORES = 8
NB = B // NCORES          # batches per core
TOK = NB * N              # tokens per core
KN = 128                  # compacted key slots per batch
KTOK = NB * KN            # compacted key tokens per core
BF16 = ml_dtypes.bfloat16

_cache = {}


def _build_nc():
    import concourse.bass as bass
    import concourse.tile as tile
    from concourse import bacc, mybir

    f32 = mybir.dt.float32
    bf16 = mybir.dt.bfloat16
    AF = mybir.ActivationFunctionType
    AOT = mybir.AluOpType

    nc = bacc.Bacc()
    xt_d = nc.declare_dram_parameter("xt", [C, TOK], bf16, isOutput=False)
    xtk_d = nc.declare_dram_parameter("xtk", [C, KTOK], bf16, isOutput=False)
    wqkv_d = nc.declare_dram_parameter("wqkv", [C, 3 * C], bf16, isOutput=False)
    qkb_d = nc.declare_dram_parameter("qkb", [128, 12], f32, isOutput=False)
    wproj_d = nc.declare_dram_parameter("wproj", [C, C], bf16, isOutput=False)
    pjb_d = nc.declare_dram_parameter("pjb", [128, 6], f32, isOutput=False)
    biasK_d = nc.declare_dram_parameter("biasK", [128, NB * H * N], bf16,
                                        isOutput=False)
    maskp_d = nc.declare_dram_parameter("maskp", [128, NB], f32, isOutput=False)
    out_d = nc.declare_dram_parameter("out", [C, TOK], f32, isOutput=True)

    NCH = [(i * 392, 392) for i in range(4)]  # token chunks for q projection
    KCH = [(0, 512), (512, 512)]              # key-token chunks for k proj

    with tile.TileContext(nc) as tc:
        from contextlib import ExitStack
        with ExitStack() as ctx:
            p_w = ctx.enter_context(tc.tile_pool(name="w", bufs=1))
            p_xt = ctx.enter_context(tc.tile_pool(name="xt", bufs=6))
            p_qk = ctx.enter_context(tc.tile_pool(name="qk", bufs=24))
            p_vx = ctx.enter_context(tc.tile_pool(name="vx", bufs=8))
            p_small = ctx.enter_context(tc.tile_pool(name="small", bufs=8))
            p_sm = ctx.enter_context(tc.tile_pool(name="sm", bufs=6))
            p_cc = ctx.enter_context(tc.tile_pool(name="cc", bufs=12))
            p_ot = ctx.enter_context(tc.tile_pool(name="ot", bufs=4))
            pp = ctx.enter_context(tc.tile_pool(name="psum", bufs=8, space="PSUM"))

            # ---- persistent inputs (split DMAs so compute starts early) ----
            maskp = p_small.tile([128, NB], f32, tag="maskp")
            nc.sync.dma_start(maskp[:], maskp_d[:])
            wq = [p_w.tile([128, 3 * C], bf16, tag="wq", bufs=6, name=f"wq{c}")
                  for c in range(6)]
            xt = [p_xt.tile([128, TOK], bf16, tag="xt", bufs=6, name=f"xt{c}")
                  for c in range(6)]
            xtk = [p_xt.tile([128, KTOK], bf16, tag="xtk", bufs=6,
                             name=f"xtk{c}") for c in range(6)]
            for c in range(6):
                nc.sync.dma_start(wq[c][:, 2 * C:3 * C],
                                  wqkv_d[c * 128:(c + 1) * 128, 2 * C:3 * C])
                nc.scalar.dma_start(xtk[c][:, :],
                                    xtk_d[c * 128:(c + 1) * 128, :])
            for c in range(6):
                nc.sync.dma_start(wq[c][:, 0:C], wqkv_d[c * 128:(c + 1) * 128, 0:C])
                nc.scalar.dma_start(xt[c][:, 0:784],
                                    xt_d[c * 128:(c + 1) * 128, 0:784])
            for c in range(6):
                nc.sync.dma_start(wq[c][:, C:2 * C],
                                  wqkv_d[c * 128:(c + 1) * 128, C:2 * C])
                nc.sync.dma_start(xt[c][:, 784:TOK],
                                  xt_d[c * 128:(c + 1) * 128, 784:TOK])
            qkb = p_small.tile([128, 12], f32, tag="qkb")
            nc.sync.dma_start(qkb[:], qkb_d[:])
            pjb = p_small.tile([128, 6], f32, tag="pjb")
            nc.sync.dma_start(pjb[:], pjb_d[:])
            biasK = []
            for b in range(NB):
                t = p_w.tile([128, H * N], bf16, tag="biasK", bufs=NB,
                             name=f"biasK{b}")
                nc.sync.dma_start(t[:], biasK_d[:, b * H * N:(b + 1) * H * N])
                biasK.append(t)
            wp = []
            for c in range(6):
                t = p_w.tile([128, C], bf16, tag="wp", bufs=6)
                nc.sync.dma_start(t[:], wproj_d[c * 128:(c + 1) * 128, :])
                wp.append(t)
            ones12 = p_small.tile([128, 12], bf16, tag="ones12")
            nc.vector.memset(ones12[:], 1.0)

            # ---- v projection (compacted keys, 65-strided heads + ones col) --
            # v bias is folded into the proj bias on the host (softmax rows
            # sum to one), so no rank-1 bias matmul here.
            vx = []
            for b in range(NB):
                vt = p_vx.tile([128, H * 65], bf16, tag="vx", bufs=8)
                ones_cols = vt[:, :].rearrange("p (h e) -> p h e", e=65)[:, :, 64:65]
                nc.scalar.activation(
                    ones_cols, ones12[:, :].rearrange("p (h e) -> p h e", e=1),
                    AF.Copy, scale=maskp[:, b:b + 1])
                for o0, hoff in ((0, 0), (384, 6)):
                    ps = pp.tile([128, 392], f32, tag="ps", bufs=3)
                    for c in range(6):
                        nc.tensor.matmul(
                            ps[:, :384],
                            xtk[c][:, b * KN:(b + 1) * KN],
                            wq[c][:, 2 * C + o0: 2 * C + o0 + 384],
                            start=(c == 0), stop=(c == 5),
                        )
                    dst = vt[:, hoff * 65:(hoff + 6) * 65].rearrange(
                        "p (h e) -> p h e", e=65)[:, :, 0:64]
                    src = ps[:, :384].rearrange("p (h e) -> p h e", e=64)
                    if hoff == 0:
                        nc.scalar.activation(
                            dst, src, AF.Copy, scale=maskp[:, b:b + 1])
                    else:
                        nc.vector.tensor_scalar(
                            dst, src, maskp[:, b:b + 1], None, op0=AOT.mult)
                vx.append(vt)

            # ---- q projection: qkq[j][ch] [128, 392] = (Wqkv^T)^T x^T ----
            qkq = [[None] * 4 for _ in range(6)]
            def qproj(j):
                for chi, (t0, tw) in enumerate(NCH):
                    ps = pp.tile([128, 512], f32, tag="pj", bufs=2)
                    for c in range(6):
                        nc.tensor.matmul(
                            ps[:, :tw],
                            wq[c][:, j * 128:(j + 1) * 128],
                            xt[c][:, t0:t0 + tw],
                            start=(c == 0), stop=(c == 5),
                        )
                    qt = p_qk.tile([128, 392], bf16, tag="qkq", bufs=24)
                    nc.scalar.activation(qt[:, :tw], ps[:, :tw], AF.Identity,
                                         bias=qkb[:, j:j + 1], scale=1.0)
                    qkq[j][chi] = qt

            # ---- k projection over compacted keys: qkk[j][kc] [128, 512] ----
            qkk = [[None] * 2 for _ in range(6)]
            def kproj(j):
                for kc, (t0, tw) in enumerate(KCH):
                    ps = pp.tile([128, 512], f32, tag="pj", bufs=2)
                    for c in range(6):
                        nc.tensor.matmul(
                            ps[:, :tw],
                            wq[c][:, (6 + j) * 128:(7 + j) * 128],
                            xtk[c][:, t0:t0 + tw],
                            start=(c == 0), stop=(c == 5),
                        )
                    kt = p_qk.tile([128, 512], bf16, tag="qkk", bufs=12)
                    nc.scalar.activation(kt[:, :tw], ps[:, :tw], AF.Identity,
                                         bias=qkb[:, 6 + j:7 + j], scale=1.0)
                    qkk[j][kc] = kt

            # ---- attention: software-pipelined scores -> AV ----
            # The PE queue is in-order; emitting each head's AV matmul right
            # after its score matmul stalls PE on the ACT exp + DVE mul chain.
            # Scores run 2 calls ahead of their AV matmuls.
            def attn_scores(b, jq):
                chb = b // 4           # which 512-col k tile holds batch b
                off = (b % 4) * KN
                tb = (b % 2) * N       # token offset inside the 392-chunk
                chq = b // 2           # which 392-chunk holds batch b
                pt = p_sm.tile([128, 2 * N], bf16, tag="pt", bufs=8)
                for hi in range(2):
                    h = 2 * jq + hi
                    po = hi * 64
                    qAP = qkq[jq][chq][po:po + 64, tb:tb + N]
                    kAP = qkk[jq][chb][po:po + 64, off:off + KN]
                    ps_s = pp.tile([128, N], f32, tag="ps", bufs=3)
                    nc.tensor.matmul(ps_s[:, :], kAP, qAP, start=True, stop=True)
                    p0 = p_sm.tile([128, N], bf16, tag="p0", bufs=8)
                    nc.scalar.activation(p0[:, :], ps_s[:, :], AF.Exp)
                    # pt = exp(s) * exp(bias); padded slots have bias 0
                    nc.vector.tensor_mul(pt[:, hi * N:(hi + 1) * N], p0[:, :],
                                         biasK[b][:, h * N:(h + 1) * N])
                return (b, jq, pt)

            def attn_av(st, cc):
                b, jq, pt = st
                bi = b % 2
                ps_o = pp.tile([65, 2 * N], f32, tag="pso", bufs=3)
                for hi in range(2):
                    h = 2 * jq + hi
                    vsl = vx[b][:, h * 65:h * 65 + 65]
                    nc.tensor.matmul(ps_o[:, hi * N:(hi + 1) * N], vsl,
                                     pt[:, hi * N:(hi + 1) * N],
                                     start=True, stop=True)
                den = p_sm.tile([1, 2 * N], f32, tag="den", bufs=4)
                nc.any.tensor_copy(den[:, :], ps_o[64:65, :])
                rec = p_sm.tile([1, 2 * N], f32, tag="rec", bufs=4)
                nc.vector.reciprocal_approx_fast(rec[:, :], den[:, :])
                rb = p_sm.tile([64, 2 * N], f32, tag="rb", bufs=4)
                nc.gpsimd.partition_broadcast(rb[:, :], rec[:, :])
                for hi in range(2):
                    nc.vector.tensor_mul(
                        cc[jq][hi * 64:hi * 64 + 64, bi * N:(bi + 1) * N],
                        ps_o[0:64, hi * N:(hi + 1) * N],
                        rb[:, hi * N:(hi + 1) * N])

            pend = []

            def attention(b, jq, cc):
                pend.append((attn_scores(b, jq), cc))
                if len(pend) > 2:
                    st, c = pend.pop(0)
                    attn_av(st, c)

            def flush():
                while pend:
                    st, c = pend.pop(0)
                    attn_av(st, c)

            def proj_group(bp, cc, o2):
                # one output block (128 features) of the out projection for
                # batch pair bp; interleaved into later attention calls so
                # the PE has independent work while softmax chains drain
                w = 2 * N
                ps = pp.tile([128, 512], f32, tag="pj", bufs=2)
                for c2 in range(6):
                    nc.tensor.matmul(ps[:, :w],
                                     wp[c2][:, o2 * 128:(o2 + 1) * 128],
                                     cc[c2][:, 0:w],
                                     start=(c2 == 0), stop=(c2 == 5))
                ot = p_ot.tile([128, 2 * N], f32, tag="ot", bufs=4)
                nc.scalar.activation(ot[:, :w], ps[:, :w], AF.Identity,
                                     bias=pjb[:, o2:o2 + 1], scale=1.0)
                nc.sync.dma_start(
                    out_d[o2 * 128:(o2 + 1) * 128,
                          (2 * bp) * N:(2 * bp) * N + w],
                    ot[:, :w])

            def proj(bp, cc):
                for o2 in range(6):
                    proj_group(bp, cc, o2)

            def make_cc():
                cc = []
                for j in range(6):
                    cct = p_cc.tile([128, 2 * N], bf16, tag="cc", bufs=12,
                                    name=f"cct{j}")
                    cc.append(cct)
                return cc

            cc0 = make_cc()
            cc1 = make_cc()
            # 4 attention calls per (q,k)-projection iteration: the qk-proj
            # matmuls give the PE independent work while the softmax chains
            # drain, so as much attention as possible lives in this loop
            for jq in range(6):
                qproj(jq)
                kproj(jq)
                attention(0, jq, cc0)
                attention(1, jq, cc0)
                attention(2, jq, cc1)
                attention(3, jq, cc1)
            flush()
            # cc0 and cc1 are both complete here: spread their proj groups
            # one per attention call so chains always drain under PE work
            cc2 = make_cc()
            for jq in range(6):
                attention(4, jq, cc2)
                proj_group(0, cc0, jq)
            for jq in range(6):
                attention(5, jq, cc2)
                proj_group(1, cc1, jq)
            flush()
            # spread proj(2)'s six groups across BOTH final batches (one per
            # two calls) so batch 7's chains also drain under PE filler
            cc3 = make_cc()
            gi = 0
            for idx, (b, jq) in enumerate((b, jq) for b in (6, 7)
                                          for jq in range(6)):
                attention(b, jq, cc3)
                if idx % 2 == 1:
                    proj_group(2, cc2, gi)
                    gi += 1
            flush()
            proj(3, cc3)

    nc.finalize()
    return nc


def _prep_in_maps(x, qkv_w, qkv_b, proj_w, proj_b, rpe_table, rpe_index, mask):
    x = np.asarray(x, np.float32)
    qkv_w = np.asarray(qkv_w, np.float32)
    qkv_b = np.asarray(qkv_b, np.float32)
    proj_w = np.asarray(proj_w, np.float32)
    proj_b = np.asarray(proj_b, np.float32)
    rpe_table = np.asarray(rpe_table, np.float32)
    rpe_index = np.asarray(rpe_index)
    mask = np.asarray(mask).astype(bool)
    assert int(mask.sum(axis=1).max()) <= KN, "key compaction capacity exceeded"

    wqkv = qkv_w.T.copy()              # [C, 3C]
    wqkv[:, :C] *= SCALE               # fold q scaling
    wqkv = np.ascontiguousarray(wqkv).astype(BF16)
    qkb_full = qkv_b.copy()
    qkb_full[:C] *= SCALE
    qkb = np.ascontiguousarray(qkb_full[:2 * C].reshape(12, 128).T).astype(np.float32)
    wproj = np.ascontiguousarray(proj_w.T).astype(BF16)
    # v bias folded here: softmax rows sum to 1, so +vb before proj is exact
    pjb_full = proj_b + proj_w @ qkv_b[2 * C:]
    pjb = np.ascontiguousarray(pjb_full.reshape(6, 128).T).astype(np.float32)

    # relative position bias [H, n, m], exponentiated (applied
    # multiplicatively after exp); gathered per batch on compacted keys
    bias_hnm = rpe_table[rpe_index].reshape(N, N, H).transpose(2, 0, 1)  # [H,n,m]
    ebias = np.exp(bias_hnm)                                             # [H,n,m]

    in_maps = []
    for i in range(NCORES):
        xs = x[i * NB:(i + 1) * NB].reshape(TOK, C)
        xt = np.ascontiguousarray(xs.T).astype(BF16)
        msk = mask[i * NB:(i + 1) * NB]
        xtk = np.zeros((C, KTOK), np.float32)
        mk = np.zeros((128, NB), np.float32)
        bK = np.zeros((128, NB * H * N), np.float32)
        for b in range(NB):
            keys = np.nonzero(msk[b])[0]
            cnt = len(keys)
            xb = x[i * NB + b]                     # [N, C]
            xtk[:, b * KN:b * KN + cnt] = xb[keys].T
            mk[:cnt, b] = 1.0
            # bK rows = key slot, cols = (h, n): exp(bias[h, n, key])
            g = ebias[:, :, keys]                  # [H, N, cnt]
            bK[:cnt, b * H * N:(b + 1) * H * N] = (
                g.transpose(2, 0, 1).reshape(cnt, H * N))
        in_maps.append({
            "xt": xt, "xtk": np.ascontiguousarray(xtk).astype(BF16),
            "wqkv": wqkv, "qkb": qkb, "wproj": wproj, "pjb": pjb,
            "biasK": np.ascontiguousarray(bK).astype(BF16),
            "maskp": np.ascontiguousarray(mk),
        })
    return in_maps


def _run(in_maps, trace=False, tmpdir=None):
    import sys, types
    # antenv.axon_hooks is absent on this image; rebuild the NTFF hook shim
    if trace and 'antenv.axon_hooks' not in sys.modules:
        try:
            import trn_agent_boot.trn_boot as tb
            hook = tb._ntff_profile_via_ctypes('/opt/axon/libaxon_pjrt.so')
            mod = types.ModuleType('antenv.axon_hooks')
            mod.get_axon_ntff_profile_hook = lambda: hook
            import antenv
            antenv.axon_hooks = mod
            sys.modules['antenv.axon_hooks'] = mod
            import concourse.bass_utils as bu
            bu.upload_artifacts = lambda d: d
        except Exception:
            trace = False
    from concourse.bass_utils import run_bass_kernel_spmd
    if 'nc' not in _cache:
        _cache['nc'] = _build_nc()
    return run_bass_kernel_spmd(_cache['nc'], in_maps, list(range(NCORES)),
                                trace=trace, tmpdir=tmpdir)


def kernel(x, qkv_w, qkv_b, proj_w, proj_b, rpe_table, rpe_index, mask):
    in_maps = _prep_in_maps(x, qkv_w, qkv_b, proj_w, proj_b, rpe_table,
                            rpe_index, mask)
    res = _run(in_maps, trace=False)
    out = np.empty((B, N, C), np.float32)
    for i in range(NCORES):
        oc = res.results[i]["out"]            # [C, TOK]
        out[i * NB:(i + 1) * NB] = oc.T.reshape(NB, N, C)
    return out
